# revision 3
# baseline (speedup 1.0000x reference)
"""2-layer GAT + mean-pool + log_softmax on 8 TRN2 NeuronCores — single launch.

Design (dst-sharded, band-ELL layout, super-group batched):
  - T1 = [s_src(4)|s_dst(4)|h1(64)] bf16 rows built sharded (12544/core,
    matmul with per-chunk stationary xT so rows come out node-major, no
    transposes), AllGathered to a full node-indexed table.
  - Each core owns 12500 dst nodes, degree-sorted into 98 bands of 128
    (one dst per partition, edges along free dim). Bands are batched into
    equal-width super-groups (SG x L <= 96 slots) so each op covers several
    bands in ONE instruction. Per group: one indirect DMA gathers
    [128, SG*L, 72] source rows; softmax is exp(lrelu(a+b)) = max(exp(y),
    exp(0.2y)); aggregation is a trailing-axis tensor_reduce.
  - Layer-1 output rows accumulate into an SBUF-resident T2 slot tile; ONE
    bulk indirect scatter (by node id) + AllGather feeds the same band
    pipeline for layer 2 and a 64-graph one-hot pooling matmul in PSUM.
  - Host: sum 8 partial pools, mean, +b2, log_softmax.
Pad slots gather table row `n` which holds s_src=-100 => exp ~ e^-20 ~ 0.
"""
import contextlib
import os
import numpy as np
import ml_dtypes

import jax
from jax.sharding import Mesh, PartitionSpec, NamedSharding
from jax.experimental.shard_map import shard_map

import concourse.bass as bass
import concourse.bacc as bacc
import concourse.mybir as mybir
import concourse.tile as tile
from concourse.bass2jax import _bass_exec_p, install_neuronx_cc_hook, partition_id_tensor

DT = mybir.dt
AF = mybir.ActivationFunctionType
OP = mybir.AluOpType
AX = mybir.AxisListType
BF16 = ml_dtypes.bfloat16
P = 128
NEG = 0.2
SLOT_BUDGET = 96
SG_MAX = 8

FULL = dict(n=100000, ncores=8, npc=12500, nband=98, shard=12544, ngraph=64)


class _PhaseStop(Exception):
    pass


# ---------------------------------------------------------------- host prep
def prep_edges(edge_index, cfg):
    n, ncores, npc, nband = cfg["n"], cfg["ncores"], cfg["npc"], cfg["nband"]
    pad_row = n
    src = np.asarray(edge_index[0], dtype=np.int64)
    dst = np.asarray(edge_index[1], dtype=np.int64)
    loop = np.arange(n, dtype=np.int64)
    src = np.concatenate([src, loop])
    dst = np.concatenate([dst, loop])

    core = dst // npc
    ldst = dst - core * npc

    deg = np.zeros((ncores, npc), dtype=np.int64)
    np.add.at(deg, (core, ldst), 1)

    nslot = nband * P
    perm = np.full((ncores, nslot), -1, dtype=np.int64)
    slot_of = np.zeros((ncores, npc), dtype=np.int64)
    for c in range(ncores):
        order = np.argsort(-deg[c], kind="stable")
        perm[c, :npc] = order
        slot_of[c, order] = np.arange(npc)

    degs_sorted = np.zeros((ncores, nslot), dtype=np.int64)
    for c in range(ncores):
        degs_sorted[c, :npc] = deg[c, perm[c, :npc]]
    band_max = degs_sorted.reshape(ncores, nband, P).max(axis=2)
    L = np.maximum(band_max.max(axis=0), 1).astype(np.int64)

    # adaptive super-groups: consecutive bands, equal width L[t0],
    # SG*L <= SLOT_BUDGET (L is non-increasing so L[t0] is the max)
    groups = []  # (t0, sg, lg)
    t0 = 0
    while t0 < nband:
        lg = int(L[t0])
        sg = 1
        while (t0 + sg < nband and sg < SG_MAX
               and (sg + 1) * lg <= SLOT_BUDGET):
            sg += 1
        groups.append((t0, sg, lg))
        L[t0:t0 + sg] = lg
        t0 += sg

    off = np.concatenate([[0], np.cumsum(L)[:-1]])
    SL = int(L.sum())

    srcidx = np.full((ncores, P, SL), pad_row, dtype=np.int32)
    slot = slot_of[core, ldst]
    band = slot // P
    part = slot % P
    key = core * nslot + slot
    ordk = np.argsort(key, kind="stable")
    key_s = key[ordk]
    starts = np.flatnonzero(np.r_[True, key_s[1:] != key_s[:-1]])
    reps = np.diff(np.r_[starts, len(key_s)])
    run = np.arange(len(key_s)) - np.repeat(starts, reps)
    col = np.empty(len(key_s), dtype=np.int64)
    col[ordk] = off[band[ordk]] + run
    srcidx[core, part, col] = src.astype(np.int32)

    dstid = np.full((ncores, P, nband), pad_row, dtype=np.int32)
    t2loc = np.full((ncores, P, nband), npc, dtype=np.int32)
    for c in range(ncores):
        valid = perm[c] >= 0
        g = np.where(valid, perm[c] + c * npc, pad_row)
        dstid[c] = g.reshape(nband, P).T
        t2 = np.where(valid, perm[c], npc)
        t2loc[c] = t2.reshape(nband, P).T

    return dict(srcidx=srcidx, dstid=dstid, t2loc=t2loc,
                L=[int(x) for x in L], off=[int(x) for x in off], SL=SL,
                groups=groups)


def build_weights(W1, a1s, a1d, W2, a2s, a2d):
    W1T = np.asarray(W1, np.float32).T          # [F_in, 64]
    fin = W1T.shape[0]
    wf = np.zeros((fin, 80), np.float32)
    for h in range(4):
        wf[:, h] = W1T[:, 16 * h:16 * (h + 1)] @ np.asarray(a1s, np.float32)[h]
        wf[:, 4 + h] = W1T[:, 16 * h:16 * (h + 1)] @ np.asarray(a1d, np.float32)[h]
    wf[:, 8:72] = W1T
    W2T = np.asarray(W2, np.float32).T          # [64, 10]
    w2c = np.zeros((64, 16), np.float32)
    w2c[:, 0] = W2T @ np.asarray(a2s, np.float32).reshape(-1)
    w2c[:, 1] = W2T @ np.asarray(a2d, np.float32).reshape(-1)
    w2c[:, 2:12] = W2T
    return wf, w2c


# ---------------------------------------------------------------- module
def build_module(cfg, ep, use_cc=True, has_b1=True, phases="ABCDE"):
    n, ncores, npc = cfg["n"], cfg["ncores"], cfg["npc"]
    nband, shard, ng = cfg["nband"], cfg["shard"], cfg["ngraph"]
    L, off, SL, groups = ep["L"], ep["off"], ep["SL"], ep["groups"]
    nt = shard * ncores
    pad_row = n
    nc = bacc.Bacc("TRN2", target_bir_lowering=False,
                   num_devices=ncores if use_cc else 1)

    xts = nc.dram_tensor("xts", [P, shard], DT.bfloat16, kind="ExternalInput")
    wf = nc.dram_tensor("wf", [P, 80], DT.bfloat16, kind="ExternalInput")
    w2c = nc.dram_tensor("w2c", [64, 16], DT.bfloat16, kind="ExternalInput")
    srci = nc.dram_tensor("srci", [P, SL], DT.int32, kind="ExternalInput")
    dsti = nc.dram_tensor("dsti", [P, nband], DT.int32, kind="ExternalInput")
    t2li = nc.dram_tensor("t2li", [P, nband], DT.int32, kind="ExternalInput")
    bli = nc.dram_tensor("bli", [P, nband], DT.bfloat16, kind="ExternalInput")
    iog = nc.dram_tensor("iog", [P, ng], DT.bfloat16, kind="ExternalInput")
    b1i = nc.dram_tensor("b1i", [P, 64], DT.float32, kind="ExternalInput")
    idi = nc.dram_tensor("idi", [P, P], DT.float32, kind="ExternalInput")
    pool = nc.dram_tensor("pool", [ng, 12], DT.float32, kind="ExternalOutput")

    aspace = "Shared" if (use_cc and ncores > 4) else "Local"
    t1s = nc.dram_tensor("t1s", [shard, 72], DT.bfloat16, kind="Internal")
    t1f = nc.dram_tensor("t1f", [nt, 72], DT.bfloat16, kind="Internal",
                         addr_space=aspace)
    t2s = nc.dram_tensor("t2s", [npc + P, 16], DT.bfloat16, kind="Internal")
    t2f = nc.dram_tensor("t2f", [nt, 16], DT.bfloat16, kind="Internal",
                         addr_space=aspace)
    cc_groups = [list(range(ncores))]

    with tile.TileContext(nc) as tc:
        with (
            tc.tile_pool(name="cp", bufs=1) as cp,
            tc.tile_pool(name="sb", bufs=3) as sb,
            tc.tile_pool(name="pp", bufs=2, space="PSUM") as pp,
            tc.tile_pool(name="pq", bufs=1, space="PSUM") as pq,
        ):
            with contextlib.suppress(_PhaseStop):
                # ---- consts
                wfs = cp.tile([P, 80], DT.bfloat16)
                nc.sync.dma_start(wfs[:], wf[:, :])
                w2cs = cp.tile([64, 16], DT.bfloat16)
                nc.sync.dma_start(w2cs[:], w2c[:, :])
                srcis = cp.tile([P, SL], DT.int32)
                nc.sync.dma_start(srcis[:], srci[:, :])
                dstis = cp.tile([P, nband], DT.int32)
                nc.sync.dma_start(dstis[:], dsti[:, :])
                t2lis = cp.tile([P, nband], DT.int32)
                nc.sync.dma_start(t2lis[:], t2li[:, :])
                blis = cp.tile([P, nband], DT.bfloat16)
                nc.sync.dma_start(blis[:], bli[:, :])
                iogs = cp.tile([P, ng], DT.bfloat16)
                nc.sync.dma_start(iogs[:], iog[:, :])
                b1s = cp.tile([P, 64], DT.float32)
                nc.sync.dma_start(b1s[:], b1i[:, :])
                ids = cp.tile([P, P], DT.float32)
                nc.sync.dma_start(ids[:], idi[:, :])

                # ---- phase A: T1 shard build, node-major via stationary xT
                if "A" in phases:
                    xall = cp.tile([P, shard], DT.bfloat16)
                    nc.sync.dma_start(xall[:], xts[:, :])
                    nch = shard // P          # node chunks of 128
                    QB = 4                    # chunks per psum bank / write
                    for q0 in range(0, nch, QB):
                        qn = min(QB, nch - q0)
                        psA = pp.tile([P, QB, 80], DT.float32, tag="psA")
                        for qi in range(qn):
                            c0 = (q0 + qi) * P
                            nc.tensor.matmul(
                                out=psA[:, qi, :],
                                lhsT=xall[:, c0:c0 + P], rhs=wfs[:],
                                start=True, stop=True)
                        tb = sb.tile([P, QB, 72], DT.bfloat16, tag="tb")
                        nc.vector.tensor_copy(tb[:, 0:qn, :],
                                              psA[:, 0:qn, 0:72])
                        nc.sync.dma_start(
                            t1s[q0 * P:(q0 + qn) * P, :]
                            .rearrange("(q p) r -> p q r", q=qn),
                            tb[:, 0:qn, :])

                # ---- phase B: AllGather T1 + pad-row patch
                if "B" not in phases:
                    raise _PhaseStop
                if use_cc:
                    nc.gpsimd.collective_compute(
                        "AllGather", OP.bypass, replica_groups=cc_groups,
                        ins=[t1s[:, :]], outs=[t1f[:, :]])
                else:
                    for i in range(ncores):
                        nc.sync.dma_start(t1f[i * shard:(i + 1) * shard, :],
                                          t1s[:, :])
                prt = cp.tile([1, 4], DT.bfloat16)
                nc.vector.memset(prt[:], -100.0)
                nc.sync.dma_start(t1f[pad_row:pad_row + 1, 0:4], prt[:])

                # ---- phase C: layer-1 edge pipeline per super-group
                if "C" not in phases:
                    raise _PhaseStop
                sd1 = cp.tile([P, nband, 72], DT.bfloat16)
                for t in range(nband):
                    nc.gpsimd.indirect_dma_start(
                        out=sd1[:, t, :], out_offset=None, in_=t1f[:, :],
                        in_offset=bass.IndirectOffsetOnAxis(
                            ap=dstis[:, t:t + 1], axis=0))
                sd1f = cp.tile([P, nband, 4], DT.float32)
                nc.vector.tensor_copy(sd1f[:], sd1[:, :, 4:8])
                t2all = cp.tile([P, nband, 16], DT.bfloat16)
                nc.vector.memset(t2all[:], 0.0)

                for (t0, sg, lg) in groups:
                    S = sg * lg
                    o0 = off[t0]
                    g = sb.tile([P, sg, lg, 72], DT.bfloat16, tag="g1")
                    gf = g[:].rearrange("p b l r -> p (b l) r")
                    for j in range(S):
                        nc.gpsimd.indirect_dma_start(
                            out=gf[:, j, :], out_offset=None, in_=t1f[:, :],
                            in_offset=bass.IndirectOffsetOnAxis(
                                ap=srcis[:, o0 + j:o0 + j + 1], axis=0))
                    et = sb.tile([P, sg, 4, lg], DT.float32, tag="et")
                    nc.vector.tensor_tensor(
                        out=et[:],
                        in0=g[:, :, :, 0:4].rearrange("p b l h -> p b h l"),
                        in1=sd1f[:, t0:t0 + sg, :, None]
                            .to_broadcast([P, sg, 4, lg]),
                        op=OP.add)
                    e1 = sb.tile([P, sg, 4, lg], DT.float32, tag="e1")
                    nc.scalar.activation(
                        e1[:].rearrange("p b h l -> p (b h l)"),
                        et[:].rearrange("p b h l -> p (b h l)"), AF.Exp)
                    e2 = sb.tile([P, sg, 4, lg], DT.float32, tag="e2")
                    nc.scalar.activation(
                        e2[:].rearrange("p b h l -> p (b h l)"),
                        et[:].rearrange("p b h l -> p (b h l)"), AF.Exp,
                        scale=NEG)
                    p = sb.tile([P, sg, 4, lg], DT.bfloat16, tag="p1")
                    nc.vector.tensor_tensor(out=p[:], in0=e1[:], in1=e2[:],
                                            op=OP.max)
                    m = sb.tile([P, sg, 4, 16, lg], DT.bfloat16, tag="m1")
                    nc.vector.tensor_tensor(
                        out=m[:],
                        in0=g[:, :, :, 8:72]
                            .rearrange("p b l (h c) -> p b h c l", c=16),
                        in1=p[:, :, :, None, :]
                            .to_broadcast([P, sg, 4, 16, lg]),
                        op=OP.mult)
                    u = sb.tile([P, sg, 4, 16], DT.float32, tag="u1")
                    nc.vector.tensor_reduce(u[:], m[:], axis=AX.X, op=OP.add)
                    d = sb.tile([P, sg, 4], DT.float32, tag="d1")
                    nc.vector.tensor_reduce(d[:], p[:], axis=AX.X, op=OP.add)
                    nc.vector.tensor_scalar_add(d[:], d[:], 1e-16)
                    r = sb.tile([P, sg, 4], DT.float32, tag="r1")
                    nc.vector.reciprocal(r[:], d[:])
                    o = sb.tile([P, sg, 64], DT.float32, tag="o1")
                    nc.vector.tensor_tensor(
                        out=o[:].rearrange("p b (h c) -> p b h c", c=16),
                        in0=u[:],
                        in1=r[:, :, :, None].to_broadcast([P, sg, 4, 16]),
                        op=OP.mult)
                    if has_b1:
                        nc.vector.tensor_tensor(
                            out=o[:], in0=o[:],
                            in1=b1s[:, None, :].to_broadcast([P, sg, 64]),
                            op=OP.add)
                    xm = sb.tile([P, sg, 64], DT.float32, tag="xm")
                    nc.vector.tensor_scalar_min(xm[:], o[:], 0.0)
                    xe = sb.tile([P, sg, 64], DT.float32, tag="xe")
                    nc.scalar.activation(
                        xe[:].rearrange("p b c -> p (b c)"),
                        xm[:].rearrange("p b c -> p (b c)"), AF.Exp)
                    xr = sb.tile([P, sg, 64], DT.float32, tag="xr")
                    nc.scalar.activation(
                        xr[:].rearrange("p b c -> p (b c)"),
                        o[:].rearrange("p b c -> p (b c)"), AF.Relu)
                    o1 = sb.tile([P, sg, 64], DT.float32, tag="o1f")
                    nc.vector.tensor_tensor(out=o1[:], in0=xe[:], in1=xr[:],
                                            op=OP.add)
                    # T2 rows: transpose each band's [128, 64], matmul w2c
                    t2p = pp.tile([P, SG_MAX, 16], DT.float32, tag="t2p")
                    for b0 in range(0, sg, 4):
                        bn = min(4, sg - b0)
                        pst = pp.tile([64, 4, P], DT.float32, tag="pst")
                        for bi in range(bn):
                            nc.tensor.transpose(out=pst[:, bi, :],
                                                in_=o1[:, b0 + bi, :],
                                                identity=ids[:])
                        o1t = sb.tile([64, 4, P], DT.bfloat16, tag="o1t")
                        nc.vector.tensor_scalar_add(o1t[:, 0:bn, :],
                                                    pst[:, 0:bn, :], -1.0)
                        for bi in range(bn):
                            nc.tensor.matmul(
                                out=t2p[:, b0 + bi, :],
                                lhsT=o1t[:, bi, :], rhs=w2cs[:],
                                start=True, stop=True)
                    nc.vector.tensor_copy(t2all[:, t0:t0 + sg, :],
                                          t2p[:, 0:sg, :])

                # scatter T2 rows by local node id (one band per call)
                for t in range(nband):
                    nc.gpsimd.indirect_dma_start(
                        out=t2s[:, :],
                        out_offset=bass.IndirectOffsetOnAxis(
                            ap=t2lis[:, t:t + 1], axis=0),
                        in_=t2all[:, t, :], in_offset=None)

                # ---- phase D: AllGather T2 + pad/tail patch
                if "D" not in phases:
                    raise _PhaseStop
                if use_cc:
                    nc.gpsimd.collective_compute(
                        "AllGather", OP.bypass, replica_groups=cc_groups,
                        ins=[t2s[0:npc, :]], outs=[t2f[0:ncores * npc, :]])
                else:
                    for i in range(ncores):
                        nc.sync.dma_start(t2f[i * npc:(i + 1) * npc, :],
                                          t2s[0:npc, :])
                zt = cp.tile([P, 16], DT.bfloat16)
                nc.vector.memset(zt[:], 0.0)
                r0 = pad_row
                while r0 < nt:
                    k = min(P, nt - r0)
                    nc.sync.dma_start(t2f[r0:r0 + k, :], zt[0:k, :])
                    r0 += k
                pr2 = cp.tile([1, 16], DT.bfloat16)
                nc.vector.memset(pr2[:], 0.0)
                nc.vector.memset(pr2[:, 0:2], -100.0)
                nc.sync.dma_start(t2f[pad_row:pad_row + 1, :], pr2[:])

                # ---- phase E: layer-2 edge pipeline + pooling
                if "E" not in phases:
                    raise _PhaseStop
                sd2 = cp.tile([P, nband, 16], DT.bfloat16)
                for t in range(nband):
                    nc.gpsimd.indirect_dma_start(
                        out=sd2[:, t, :], out_offset=None, in_=t2f[:, :],
                        in_offset=bass.IndirectOffsetOnAxis(
                            ap=dstis[:, t:t + 1], axis=0))
                sd2f = cp.tile([P, nband, 1], DT.float32)
                nc.vector.tensor_copy(sd2f[:], sd2[:, :, 1:2])
                pps = pq.tile([ng, 12], DT.float32)

                for (t0, sg, lg) in groups:
                    S = sg * lg
                    o0 = off[t0]
                    g2 = sb.tile([P, sg, lg, 16], DT.bfloat16, tag="g2")
                    g2f = g2[:].rearrange("p b l r -> p (b l) r")
                    for j in range(S):
                        nc.gpsimd.indirect_dma_start(
                            out=g2f[:, j, :], out_offset=None, in_=t2f[:, :],
                            in_offset=bass.IndirectOffsetOnAxis(
                                ap=srcis[:, o0 + j:o0 + j + 1], axis=0))
                    et2 = sb.tile([P, sg, lg], DT.float32, tag="et2")
                    nc.vector.tensor_tensor(
                        out=et2[:], in0=g2[:, :, :, 0],
                        in1=sd2f[:, t0:t0 + sg, :].to_broadcast([P, sg, lg]),
                        op=OP.add)
                    f1 = sb.tile([P, sg, lg], DT.float32, tag="f1")
                    nc.scalar.activation(
                        f1[:].rearrange("p b l -> p (b l)"),
                        et2[:].rearrange("p b l -> p (b l)"), AF.Exp)
                    f2 = sb.tile([P, sg, lg], DT.float32, tag="f2")
                    nc.scalar.activation(
                        f2[:].rearrange("p b l -> p (b l)"),
                        et2[:].rearrange("p b l -> p (b l)"), AF.Exp,
                        scale=NEG)
                    p2 = sb.tile([P, sg, lg], DT.bfloat16, tag="p2")
                    nc.vector.tensor_tensor(out=p2[:], in0=f1[:], in1=f2[:],
                                            op=OP.max)
                    m2 = sb.tile([P, sg, 10, lg], DT.bfloat16, tag="m2")
                    nc.vector.tensor_tensor(
                        out=m2[:],
                        in0=g2[:, :, :, 2:12].rearrange("p b l c -> p b c l"),
                        in1=p2[:, :, None, :].to_broadcast([P, sg, 10, lg]),
                        op=OP.mult)
                    u2 = sb.tile([P, sg, 10], DT.float32, tag="u2")
                    nc.vector.tensor_reduce(u2[:], m2[:], axis=AX.X, op=OP.add)
                    d2 = sb.tile([P, sg], DT.float32, tag="d2")
                    nc.vector.tensor_reduce(d2[:], p2[:], axis=AX.X, op=OP.add)
                    nc.vector.tensor_scalar_add(d2[:], d2[:], 1e-16)
                    r2 = sb.tile([P, sg], DT.float32, tag="r2")
                    nc.vector.reciprocal(r2[:], d2[:])
                    rhsp = sb.tile([P, sg, 12], DT.bfloat16, tag="rhsp")
                    nc.vector.memset(rhsp[:, :, 10:11], 1.0)
                    nc.vector.memset(rhsp[:, :, 11:12], 0.0)
                    nc.vector.tensor_tensor(
                        out=rhsp[:, :, 0:10], in0=u2[:],
                        in1=r2[:, :, None].to_broadcast([P, sg, 10]),
                        op=OP.mult)
                    sbh = sb.tile([P, sg, ng], DT.bfloat16, tag="sbh")
                    nc.vector.tensor_tensor(
                        out=sbh[:],
                        in0=blis[:, t0:t0 + sg, None].to_broadcast([P, sg, ng]),
                        in1=iogs[:, None, :].to_broadcast([P, sg, ng]),
                        op=OP.is_equal)
                    for b in range(sg):
                        tg = t0 + b
                        nc.tensor.matmul(out=pps[:], lhsT=sbh[:, b, :],
                                         rhs=rhsp[:, b, :],
                                         start=(tg == 0),
                                         stop=(tg == nband - 1),
                                         tile_position=(0, 0))

                po = cp.tile([ng, 12], DT.float32)
                nc.vector.tensor_copy(po[:], pps[:])
                nc.sync.dma_start(pool[:, :], po[:])

    nc.compile()
    return nc


# ---------------------------------------------------------------- launcher
class Launcher:
    def __init__(self, nc, n_cores):
        install_neuronx_cc_hook()
        self.nc = nc
        self.n_cores = n_cores
        pname = nc.partition_id_tensor.name if nc.partition_id_tensor else None
        in_names, out_names, out_avals, zero_outs = [], [], [], []
        for alloc in nc.m.functions[0].allocations:
            if not isinstance(alloc, mybir.MemoryLocationSet):
                continue
            name = alloc.memorylocations[0].name
            if alloc.kind == "ExternalInput":
                if name != pname:
                    in_names.append(name)
            elif alloc.kind == "ExternalOutput":
                out_names.append(name)
                shape = tuple(alloc.tensor_shape)
                dtype = mybir.dt.np(alloc.dtype)
                out_avals.append(jax.core.ShapedArray(shape, dtype))
                zero_outs.append(np.zeros(shape, dtype))
        self.in_names, self.out_names = in_names, out_names
        self.out_avals, self.zero_outs = out_avals, zero_outs
        n_params, n_outs = len(in_names), len(out_avals)
        all_in = in_names + out_names + ([pname] if pname else [])

        def _body(*args):
            operands = list(args)
            if pname is not None:
                operands.append(partition_id_tensor())
            return tuple(_bass_exec_p.bind(
                *operands, out_avals=tuple(out_avals), in_names=tuple(all_in),
                out_names=tuple(out_names), lowering_input_output_aliases=(),
                sim_require_finite=True, sim_require_nnan=True, nc=nc))

        devices = jax.devices()[:n_cores]
        self.mesh = Mesh(np.asarray(devices), ("core",))
        specs_in = (PartitionSpec("core"),) * (n_params + n_outs)
        specs_out = (PartitionSpec("core"),) * n_outs
        self.fn = jax.jit(shard_map(_body, mesh=self.mesh, in_specs=specs_in,
                                    out_specs=specs_out, check_rep=False),
                          keep_unused=True)
        self.sharding = NamedSharding(self.mesh, PartitionSpec("core"))

    def put(self, arr_percore):
        a = np.ascontiguousarray(arr_percore)
        return jax.device_put(a.reshape(a.shape[0] * a.shape[1], *a.shape[2:]),
                              self.sharding)

    def __call__(self, named_args):
        args = [named_args[n] for n in self.in_names]
        for z in self.zero_outs:
            zz = np.zeros((self.n_cores * z.shape[0], *z.shape[1:]), z.dtype)
            args.append(jax.device_put(zz, self.sharding))
        outs = self.fn(*args)
        return dict(zip(self.out_names, outs))


# ---------------------------------------------------------------- host side
_CACHE = {}


def make_inputs(x, edge_index, batch, W1, a1s, a1d, b1, W2, a2s, a2d, cfg, ep):
    n, ncores, npc = cfg["n"], cfg["ncores"], cfg["npc"]
    nband, shard, ng = cfg["nband"], cfg["shard"], cfg["ngraph"]
    nt = shard * ncores
    wf, w2c = build_weights(W1, a1s, a1d, W2, a2s, a2d)

    xtp = np.zeros((P, nt), np.float32)
    xtp[:, :n] = np.asarray(x, np.float32).T
    xtp = xtp.astype(BF16)
    xts = np.stack([xtp[:, c * shard:(c + 1) * shard] for c in range(ncores)])

    batch = np.asarray(batch, np.int64)
    dstid = ep["dstid"]
    bl = np.where(dstid < n, batch[np.minimum(dstid, n - 1)], 200).astype(BF16)

    rep = lambda a: np.broadcast_to(a, (ncores, *a.shape)).copy()
    iog = np.broadcast_to(np.arange(ng, dtype=np.float32).astype(BF16),
                          (P, ng)).copy()
    b1b = np.broadcast_to(np.asarray(b1, np.float32), (P, 64)).copy()
    ident = np.eye(P, dtype=np.float32)

    return {
        "xts": xts,
        "wf": rep(wf.astype(BF16)),
        "w2c": rep(w2c.astype(BF16)),
        "srci": ep["srcidx"],
        "dsti": ep["dstid"],
        "t2li": ep["t2loc"],
        "bli": bl,
        "iog": rep(iog),
        "b1i": rep(b1b),
        "idi": rep(ident),
    }


def finish(pool_parts, b2, ng):
    acc = pool_parts.astype(np.float64).sum(axis=0)
    sums = acc[:, :10]
    cnts = np.maximum(acc[:, 10], 1.0)
    pooled = (sums / cnts[:, None] + np.asarray(b2, np.float64)).astype(np.float32)
    m = pooled.max(axis=1, keepdims=True)
    z = pooled - m
    return (z - np.log(np.exp(z).sum(axis=1, keepdims=True))).astype(np.float32)


def kernel(x, edge_index, batch, W1, att_src1, att_dst1, b1,
           W2, att_src2, att_dst2, b2):
    cfg = FULL
    ep = prep_edges(edge_index, cfg)
    key = (tuple(ep["L"]), bool(np.any(np.asarray(b1))))
    if key not in _CACHE:
        nc = build_module(cfg, ep, use_cc=True,
                          has_b1=bool(np.any(np.asarray(b1))))
        _CACHE[key] = Launcher(nc, cfg["ncores"])
    lau = _CACHE[key]

    named = make_inputs(x, edge_index, batch, W1, att_src1, att_dst1, b1,
                        W2, att_src2, att_dst2, cfg, ep)
    named = {k: lau.put(v) for k, v in named.items()}
    outs = lau(named)
    pool = np.asarray(outs["pool"]).reshape(cfg["ncores"], cfg["ngraph"], 12)
    return finish(pool, b2, cfg["ngraph"])


# revision 6
# speedup vs baseline: 1.4064x; 1.4064x over previous
"""2-layer GAT + mean-pool + log_softmax on 8 TRN2 NeuronCores — single launch.

Design (dst-sharded, band-ELL layout, super-group batched):
  - T1 = [s_src(4)|s_dst(4)|h1(64)] bf16 rows built sharded (12544/core,
    matmul with per-chunk stationary xT so rows come out node-major, no
    transposes), AllGathered to a full node-indexed table.
  - Each core owns 12500 dst nodes, degree-sorted into 98 bands of 128
    (one dst per partition, edges along free dim). Bands are batched into
    equal-width super-groups (SG x L <= 96 slots) so each op covers several
    bands in ONE instruction. Per group: one indirect DMA gathers
    [128, SG*L, 72] source rows; softmax is exp(lrelu(a+b)) = max(exp(y),
    exp(0.2y)); aggregation is a trailing-axis tensor_reduce.
  - Layer-1 output rows accumulate into an SBUF-resident T2 slot tile; ONE
    bulk indirect scatter (by node id) + AllGather feeds the same band
    pipeline for layer 2 and a 64-graph one-hot pooling matmul in PSUM.
  - Host: sum 8 partial pools, mean, +b2, log_softmax.
Pad slots gather table row `n` which holds s_src=-100 => exp ~ e^-20 ~ 0.
"""
import contextlib
import os
import numpy as np
import ml_dtypes

import jax
from jax.sharding import Mesh, PartitionSpec, NamedSharding
from jax.experimental.shard_map import shard_map

import concourse.bass as bass
import concourse.bacc as bacc
import concourse.mybir as mybir
import concourse.tile as tile
from concourse.bass2jax import _bass_exec_p, install_neuronx_cc_hook, partition_id_tensor

DT = mybir.dt
AF = mybir.ActivationFunctionType
OP = mybir.AluOpType
AX = mybir.AxisListType
BF16 = ml_dtypes.bfloat16
P = 128
NEG = 0.2
SLOT_BUDGET = int(os.environ.get("KV_SLOTS", "96"))
SG_MAX = int(os.environ.get("KV_SGMAX", "8"))

FULL = dict(n=100000, ncores=8, npc=12500, nband=98, shard=12544, ngraph=64)


class _PhaseStop(Exception):
    pass


# ---------------------------------------------------------------- host prep
def prep_edges(edge_index, cfg):
    n, ncores, npc, nband = cfg["n"], cfg["ncores"], cfg["npc"], cfg["nband"]
    pad_row = n
    src = np.asarray(edge_index[0], dtype=np.int64)
    dst = np.asarray(edge_index[1], dtype=np.int64)
    loop = np.arange(n, dtype=np.int64)
    src = np.concatenate([src, loop])
    dst = np.concatenate([dst, loop])

    core = dst // npc
    ldst = dst - core * npc

    deg = np.zeros((ncores, npc), dtype=np.int64)
    np.add.at(deg, (core, ldst), 1)

    nslot = nband * P
    perm = np.full((ncores, nslot), -1, dtype=np.int64)
    slot_of = np.zeros((ncores, npc), dtype=np.int64)
    for c in range(ncores):
        order = np.argsort(-deg[c], kind="stable")
        perm[c, :npc] = order
        slot_of[c, order] = np.arange(npc)

    degs_sorted = np.zeros((ncores, nslot), dtype=np.int64)
    for c in range(ncores):
        degs_sorted[c, :npc] = deg[c, perm[c, :npc]]
    band_max = degs_sorted.reshape(ncores, nband, P).max(axis=2)
    L = np.maximum(band_max.max(axis=0), 1).astype(np.int64)

    # adaptive super-groups: consecutive bands, equal width L[t0],
    # SG*L <= SLOT_BUDGET (L is non-increasing so L[t0] is the max)
    Ltrue = [int(x) for x in L]   # pre-equalization widths (gather bound)
    groups = []  # (t0, sg, lg)
    t0 = 0
    while t0 < nband:
        lg = int(L[t0])
        sg = 1
        while (t0 + sg < nband and sg < SG_MAX
               and (sg + 1) * lg <= SLOT_BUDGET):
            sg += 1
        groups.append((t0, sg, lg))
        L[t0:t0 + sg] = lg
        t0 += sg

    off = np.concatenate([[0], np.cumsum(L)[:-1]])
    SL = int(L.sum())

    # slot-space row id of every node: core*nslot + slot_of(node)
    # (tables are stored in degree-sorted slot order; pad -> slot 12543 of
    # core 0, a guaranteed dummy slot patched to s=-100)
    src_core = src // npc
    src_slot = (src_core * nslot + slot_of[src_core, src - src_core * npc])
    pad_slot = nslot - 1

    srcidx = np.full((ncores, P, SL), pad_slot, dtype=np.int32)
    slot = slot_of[core, ldst]
    band = slot // P
    part = slot % P
    key = core * nslot + slot
    ordk = np.argsort(key, kind="stable")
    key_s = key[ordk]
    starts = np.flatnonzero(np.r_[True, key_s[1:] != key_s[:-1]])
    reps = np.diff(np.r_[starts, len(key_s)])
    run = np.arange(len(key_s)) - np.repeat(starts, reps)
    col = np.empty(len(key_s), dtype=np.int64)
    col[ordk] = off[band[ordk]] + run
    srcidx[core, part, col] = src_slot.astype(np.int32)

    # node id per slot (for xts permutation and batchloc); -1 for dummies
    slot_node = np.full((ncores, nslot), -1, dtype=np.int64)
    for c in range(ncores):
        valid = perm[c] >= 0
        slot_node[c] = np.where(valid, perm[c] + c * npc, -1)

    return dict(srcidx=srcidx, slot_node=slot_node,
                L=[int(x) for x in L], off=[int(x) for x in off], SL=SL,
                groups=groups, Ltrue=Ltrue)


def build_weights(W1, a1s, a1d, W2, a2s, a2d):
    W1T = np.asarray(W1, np.float32).T          # [F_in, 64]
    fin = W1T.shape[0]
    wf = np.zeros((fin, 80), np.float32)
    for h in range(4):
        wf[:, h] = W1T[:, 16 * h:16 * (h + 1)] @ np.asarray(a1s, np.float32)[h]
        wf[:, 4 + h] = W1T[:, 16 * h:16 * (h + 1)] @ np.asarray(a1d, np.float32)[h]
    wf[:, 8:72] = W1T
    W2T = np.asarray(W2, np.float32).T          # [64, 10]
    w2c = np.zeros((64, 16), np.float32)
    w2c[:, 0] = W2T @ np.asarray(a2s, np.float32).reshape(-1)
    w2c[:, 1] = W2T @ np.asarray(a2d, np.float32).reshape(-1)
    w2c[:, 2:12] = W2T
    return wf, w2c


# ---------------------------------------------------------------- module
def build_module(cfg, ep, use_cc=True, has_b1=True, phases="ABCDE"):
    n, ncores, npc = cfg["n"], cfg["ncores"], cfg["npc"]
    nband, shard, ng = cfg["nband"], cfg["shard"], cfg["ngraph"]
    L, off, SL, groups = ep["L"], ep["off"], ep["SL"], ep["groups"]
    Ltrue = ep.get("Ltrue", L)
    nt = shard * ncores
    pad_row = n
    nc = bacc.Bacc("TRN2", target_bir_lowering=False,
                   num_devices=ncores if use_cc else 1)

    xts = nc.dram_tensor("xts", [P, shard], DT.bfloat16, kind="ExternalInput")
    wf = nc.dram_tensor("wf", [P, 80], DT.bfloat16, kind="ExternalInput")
    w2c = nc.dram_tensor("w2c", [64, 16], DT.bfloat16, kind="ExternalInput")
    srci = nc.dram_tensor("srci", [P, SL], DT.int32, kind="ExternalInput")
    bli = nc.dram_tensor("bli", [P, nband], DT.bfloat16, kind="ExternalInput")
    iog = nc.dram_tensor("iog", [P, ng], DT.bfloat16, kind="ExternalInput")
    b1i = nc.dram_tensor("b1i", [P, 64], DT.float32, kind="ExternalInput")
    idi = nc.dram_tensor("idi", [P, P], DT.float32, kind="ExternalInput")
    pool = nc.dram_tensor("pool", [ng, 12], DT.float32, kind="ExternalOutput")

    aspace = "Shared" if (use_cc and ncores > 4) else "Local"
    t1s = nc.dram_tensor("t1s", [shard, 72], DT.bfloat16, kind="Internal")
    t1f = nc.dram_tensor("t1f", [nt, 72], DT.bfloat16, kind="Internal",
                         addr_space=aspace)
    t2s = nc.dram_tensor("t2s", [nband * P, 16], DT.bfloat16, kind="Internal")
    t2f = nc.dram_tensor("t2f", [nt, 16], DT.bfloat16, kind="Internal",
                         addr_space=aspace)
    cc_groups = [list(range(ncores))]

    with tile.TileContext(nc) as tc:
        with (
            tc.tile_pool(name="cp", bufs=1) as cp,
            tc.tile_pool(name="sb", bufs=3) as sb,
            tc.tile_pool(name="pp", bufs=2, space="PSUM") as pp,
            tc.tile_pool(name="pq", bufs=1, space="PSUM") as pq,
        ):
            with contextlib.suppress(_PhaseStop):
                # ---- consts
                wfs = cp.tile([P, 80], DT.bfloat16)
                nc.sync.dma_start(wfs[:], wf[:, :])
                w2cs = cp.tile([64, 16], DT.bfloat16)
                nc.sync.dma_start(w2cs[:], w2c[:, :])
                srcis = cp.tile([P, SL], DT.int32)
                nc.sync.dma_start(srcis[:], srci[:, :])
                blis = cp.tile([P, nband], DT.bfloat16)
                nc.sync.dma_start(blis[:], bli[:, :])
                iogs = cp.tile([P, ng], DT.bfloat16)
                nc.sync.dma_start(iogs[:], iog[:, :])
                b1s = cp.tile([P, 64], DT.float32)
                nc.sync.dma_start(b1s[:], b1i[:, :])
                ids = cp.tile([P, P], DT.float32)
                nc.sync.dma_start(ids[:], idi[:, :])

                # ---- phase A: T1 shard build, node-major via stationary xT
                if "A" in phases:
                    xall = cp.tile([P, shard], DT.bfloat16)
                    nc.sync.dma_start(xall[:], xts[:, :])
                    nch = shard // P          # node chunks of 128
                    QB = 4                    # chunks per psum bank / write
                    for q0 in range(0, nch, QB):
                        qn = min(QB, nch - q0)
                        psA = pp.tile([P, QB, 80], DT.float32, tag="psA")
                        for qi in range(qn):
                            c0 = (q0 + qi) * P
                            nc.tensor.matmul(
                                out=psA[:, qi, :],
                                lhsT=xall[:, c0:c0 + P], rhs=wfs[:],
                                start=True, stop=True)
                        tb = sb.tile([P, QB, 72], DT.bfloat16, tag="tb")
                        nc.vector.tensor_copy(tb[:, 0:qn, :],
                                              psA[:, 0:qn, 0:72])
                        nc.sync.dma_start(
                            t1s[q0 * P:(q0 + qn) * P, :]
                            .rearrange("(q p) r -> p q r", q=qn),
                            tb[:, 0:qn, :])

                # ---- phase B: patch pad slot in t1s, AllGather T1
                if "B" not in phases:
                    raise _PhaseStop
                prt = cp.tile([1, 4], DT.bfloat16)
                nc.vector.memset(prt[:], -100.0)
                nc.sync.dma_start(t1s[nband * P - 1:nband * P, 0:4], prt[:])
                if use_cc:
                    nc.gpsimd.collective_compute(
                        "AllGather", OP.bypass, replica_groups=cc_groups,
                        ins=[t1s[:, :]], outs=[t1f[:, :]])
                else:
                    for i in range(ncores):
                        nc.sync.dma_start(t1f[i * shard:(i + 1) * shard, :],
                                          t1s[:, :])
                # ---- phase C: layer-1 edge pipeline per super-group
                if "C" not in phases:
                    raise _PhaseStop
                sd1 = cp.tile([P, nband, 72], DT.bfloat16)
                nc.sync.dma_start(
                    sd1[:], t1s[:, :].rearrange("(t p) r -> p t r", p=P))
                sd1f = cp.tile([P, nband, 4], DT.float32)
                nc.vector.tensor_copy(sd1f[:], sd1[:, :, 4:8])
                t2all = cp.tile([P, nband, 16], DT.bfloat16)
                nc.vector.memset(t2all[:], 0.0)

                cmax = int(os.environ.get("KV_CMAX", "9999"))
                for (t0, sg, lg) in groups[:cmax]:
                    S = sg * lg
                    o0 = off[t0]
                    g = sb.tile([P, sg, lg, 72], DT.bfloat16, tag="g1")
                    gf = g[:].rearrange("p b l r -> p (b l) r")
                    for b in range(sg):
                        lt = Ltrue[t0 + b]
                        if lt < lg:
                            nc.vector.memset(g[:, b, lt:lg, :], -100.0)
                        for j in range(lt):
                            jj = b * lg + j
                            nc.gpsimd.indirect_dma_start(
                                out=gf[:, jj, :], out_offset=None,
                                in_=t1f[:, :],
                                in_offset=bass.IndirectOffsetOnAxis(
                                    ap=srcis[:, o0 + jj:o0 + jj + 1], axis=0))
                    et = sb.tile([P, sg, 4, lg], DT.float32, tag="et")
                    nc.vector.tensor_tensor(
                        out=et[:],
                        in0=g[:, :, :, 0:4].rearrange("p b l h -> p b h l"),
                        in1=sd1f[:, t0:t0 + sg, :, None]
                            .to_broadcast([P, sg, 4, lg]),
                        op=OP.add)
                    e1 = sb.tile([P, sg, 4, lg], DT.float32, tag="e1")
                    nc.scalar.activation(
                        e1[:].rearrange("p b h l -> p (b h l)"),
                        et[:].rearrange("p b h l -> p (b h l)"), AF.Exp)
                    e2 = sb.tile([P, sg, 4, lg], DT.float32, tag="e2")
                    nc.scalar.activation(
                        e2[:].rearrange("p b h l -> p (b h l)"),
                        et[:].rearrange("p b h l -> p (b h l)"), AF.Exp,
                        scale=NEG)
                    p = sb.tile([P, sg, 4, lg], DT.bfloat16, tag="p1")
                    nc.vector.tensor_tensor(out=p[:], in0=e1[:], in1=e2[:],
                                            op=OP.max)
                    m = sb.tile([P, sg, 4, 16, lg], DT.bfloat16, tag="m1")
                    nc.vector.tensor_tensor(
                        out=m[:],
                        in0=g[:, :, :, 8:72]
                            .rearrange("p b l (h c) -> p b h c l", c=16),
                        in1=p[:, :, :, None, :]
                            .to_broadcast([P, sg, 4, 16, lg]),
                        op=OP.mult)
                    u = sb.tile([P, sg, 4, 16], DT.float32, tag="u1")
                    nc.vector.tensor_reduce(u[:], m[:], axis=AX.X, op=OP.add)
                    d = sb.tile([P, sg, 4], DT.float32, tag="d1")
                    nc.vector.tensor_reduce(d[:], p[:], axis=AX.X, op=OP.add)
                    nc.vector.tensor_scalar_add(d[:], d[:], 1e-16)
                    r = sb.tile([P, sg, 4], DT.float32, tag="r1")
                    nc.vector.reciprocal(r[:], d[:])
                    o = sb.tile([P, sg, 64], DT.float32, tag="o1")
                    nc.vector.tensor_tensor(
                        out=o[:].rearrange("p b (h c) -> p b h c", c=16),
                        in0=u[:],
                        in1=r[:, :, :, None].to_broadcast([P, sg, 4, 16]),
                        op=OP.mult)
                    if has_b1:
                        nc.vector.tensor_tensor(
                            out=o[:], in0=o[:],
                            in1=b1s[:, None, :].to_broadcast([P, sg, 64]),
                            op=OP.add)
                    xm = sb.tile([P, sg, 64], DT.float32, tag="xm")
                    nc.vector.tensor_scalar_min(xm[:], o[:], 0.0)
                    xe = sb.tile([P, sg, 64], DT.float32, tag="xe")
                    nc.scalar.activation(
                        xe[:].rearrange("p b c -> p (b c)"),
                        xm[:].rearrange("p b c -> p (b c)"), AF.Exp)
                    xr = sb.tile([P, sg, 64], DT.float32, tag="xr")
                    nc.scalar.activation(
                        xr[:].rearrange("p b c -> p (b c)"),
                        o[:].rearrange("p b c -> p (b c)"), AF.Relu)
                    o1 = sb.tile([P, sg, 64], DT.float32, tag="o1f")
                    nc.vector.tensor_tensor(out=o1[:], in0=xe[:], in1=xr[:],
                                            op=OP.add)
                    # T2 rows: transpose each band's [128, 64], matmul w2c
                    t2p = pp.tile([P, SG_MAX, 16], DT.float32, tag="t2p")
                    for b0 in range(0, sg, 4):
                        bn = min(4, sg - b0)
                        pst = pp.tile([64, 4, P], DT.float32, tag="pst")
                        for bi in range(bn):
                            nc.tensor.transpose(out=pst[:, bi, :],
                                                in_=o1[:, b0 + bi, :],
                                                identity=ids[:])
                        o1t = sb.tile([64, 4, P], DT.bfloat16, tag="o1t")
                        nc.vector.tensor_scalar_add(o1t[:, 0:bn, :],
                                                    pst[:, 0:bn, :], -1.0)
                        for bi in range(bn):
                            nc.tensor.matmul(
                                out=t2p[:, b0 + bi, :],
                                lhsT=o1t[:, bi, :], rhs=w2cs[:],
                                start=True, stop=True)
                    nc.vector.tensor_copy(t2all[:, t0:t0 + sg, :],
                                          t2p[:, 0:sg, :])

                # slot-ordered write of the whole T2 shard, then patch the
                # pad slot (last dummy slot): s2 cols = -100
                nc.sync.dma_start(
                    t2s[:, :].rearrange("(t p) r -> p t r", p=P), t2all[:])
                pr2 = cp.tile([1, 2], DT.bfloat16)
                nc.vector.memset(pr2[:], -100.0)
                nc.sync.dma_start(t2s[nband * P - 1:nband * P, 0:2], pr2[:])

                # ---- phase D: AllGather T2 + pad/tail patch
                if "D" not in phases:
                    raise _PhaseStop
                if use_cc:
                    nc.gpsimd.collective_compute(
                        "AllGather", OP.bypass, replica_groups=cc_groups,
                        ins=[t2s[:, :]], outs=[t2f[:, :]])
                else:
                    for i in range(ncores):
                        nc.sync.dma_start(
                            t2f[i * nband * P:(i + 1) * nband * P, :],
                            t2s[:, :])

                # ---- phase E: layer-2 edge pipeline + pooling
                if "E" not in phases:
                    raise _PhaseStop
                sd2f = cp.tile([P, nband, 1], DT.float32)
                nc.vector.tensor_copy(sd2f[:], t2all[:, :, 1:2])
                pps = pq.tile([ng, 12], DT.float32)

                for (t0, sg, lg) in groups:
                    S = sg * lg
                    o0 = off[t0]
                    g2 = sb.tile([P, sg, lg, 16], DT.bfloat16, tag="g2")
                    g2f = g2[:].rearrange("p b l r -> p (b l) r")
                    for b in range(sg):
                        lt = Ltrue[t0 + b]
                        if lt < lg:
                            nc.vector.memset(g2[:, b, lt:lg, :], -100.0)
                        for j in range(lt):
                            jj = b * lg + j
                            nc.gpsimd.indirect_dma_start(
                                out=g2f[:, jj, :], out_offset=None,
                                in_=t2f[:, :],
                                in_offset=bass.IndirectOffsetOnAxis(
                                    ap=srcis[:, o0 + jj:o0 + jj + 1], axis=0))
                    et2 = sb.tile([P, sg, lg], DT.float32, tag="et2")
                    nc.vector.tensor_tensor(
                        out=et2[:], in0=g2[:, :, :, 0],
                        in1=sd2f[:, t0:t0 + sg, :].to_broadcast([P, sg, lg]),
                        op=OP.add)
                    f1 = sb.tile([P, sg, lg], DT.float32, tag="f1")
                    nc.scalar.activation(
                        f1[:].rearrange("p b l -> p (b l)"),
                        et2[:].rearrange("p b l -> p (b l)"), AF.Exp)
                    f2 = sb.tile([P, sg, lg], DT.float32, tag="f2")
                    nc.scalar.activation(
                        f2[:].rearrange("p b l -> p (b l)"),
                        et2[:].rearrange("p b l -> p (b l)"), AF.Exp,
                        scale=NEG)
                    p2 = sb.tile([P, sg, lg], DT.bfloat16, tag="p2")
                    nc.vector.tensor_tensor(out=p2[:], in0=f1[:], in1=f2[:],
                                            op=OP.max)
                    m2 = sb.tile([P, sg, 10, lg], DT.bfloat16, tag="m2")
                    nc.vector.tensor_tensor(
                        out=m2[:],
                        in0=g2[:, :, :, 2:12].rearrange("p b l c -> p b c l"),
                        in1=p2[:, :, None, :].to_broadcast([P, sg, 10, lg]),
                        op=OP.mult)
                    u2 = sb.tile([P, sg, 10], DT.float32, tag="u2")
                    nc.vector.tensor_reduce(u2[:], m2[:], axis=AX.X, op=OP.add)
                    d2 = sb.tile([P, sg], DT.float32, tag="d2")
                    nc.vector.tensor_reduce(d2[:], p2[:], axis=AX.X, op=OP.add)
                    nc.vector.tensor_scalar_add(d2[:], d2[:], 1e-16)
                    r2 = sb.tile([P, sg], DT.float32, tag="r2")
                    nc.vector.reciprocal(r2[:], d2[:])
                    rhsp = sb.tile([P, sg, 12], DT.bfloat16, tag="rhsp")
                    nc.vector.memset(rhsp[:, :, 10:11], 1.0)
                    nc.vector.memset(rhsp[:, :, 11:12], 0.0)
                    nc.vector.tensor_tensor(
                        out=rhsp[:, :, 0:10], in0=u2[:],
                        in1=r2[:, :, None].to_broadcast([P, sg, 10]),
                        op=OP.mult)
                    sbh = sb.tile([P, sg, ng], DT.bfloat16, tag="sbh")
                    nc.vector.tensor_tensor(
                        out=sbh[:],
                        in0=blis[:, t0:t0 + sg, None].to_broadcast([P, sg, ng]),
                        in1=iogs[:, None, :].to_broadcast([P, sg, ng]),
                        op=OP.is_equal)
                    for b in range(sg):
                        tg = t0 + b
                        nc.tensor.matmul(out=pps[:], lhsT=sbh[:, b, :],
                                         rhs=rhsp[:, b, :],
                                         start=(tg == 0),
                                         stop=(tg == nband - 1),
                                         tile_position=(0, 0))

                po = cp.tile([ng, 12], DT.float32)
                nc.vector.tensor_copy(po[:], pps[:])
                nc.sync.dma_start(pool[:, :], po[:])

    nc.compile()
    return nc


# ---------------------------------------------------------------- launcher
class Launcher:
    def __init__(self, nc, n_cores):
        install_neuronx_cc_hook()
        self.nc = nc
        self.n_cores = n_cores
        pname = nc.partition_id_tensor.name if nc.partition_id_tensor else None
        in_names, out_names, out_avals, zero_outs = [], [], [], []
        for alloc in nc.m.functions[0].allocations:
            if not isinstance(alloc, mybir.MemoryLocationSet):
                continue
            name = alloc.memorylocations[0].name
            if alloc.kind == "ExternalInput":
                if name != pname:
                    in_names.append(name)
            elif alloc.kind == "ExternalOutput":
                out_names.append(name)
                shape = tuple(alloc.tensor_shape)
                dtype = mybir.dt.np(alloc.dtype)
                out_avals.append(jax.core.ShapedArray(shape, dtype))
                zero_outs.append(np.zeros(shape, dtype))
        self.in_names, self.out_names = in_names, out_names
        self.out_avals, self.zero_outs = out_avals, zero_outs
        n_params, n_outs = len(in_names), len(out_avals)
        all_in = in_names + out_names + ([pname] if pname else [])

        def _body(*args):
            operands = list(args)
            if pname is not None:
                operands.append(partition_id_tensor())
            return tuple(_bass_exec_p.bind(
                *operands, out_avals=tuple(out_avals), in_names=tuple(all_in),
                out_names=tuple(out_names), lowering_input_output_aliases=(),
                sim_require_finite=True, sim_require_nnan=True, nc=nc))

        devices = jax.devices()[:n_cores]
        self.mesh = Mesh(np.asarray(devices), ("core",))
        specs_in = (PartitionSpec("core"),) * (n_params + n_outs)
        specs_out = (PartitionSpec("core"),) * n_outs
        self.fn = jax.jit(shard_map(_body, mesh=self.mesh, in_specs=specs_in,
                                    out_specs=specs_out, check_rep=False),
                          keep_unused=True)
        self.sharding = NamedSharding(self.mesh, PartitionSpec("core"))

    def put(self, arr_percore):
        a = np.ascontiguousarray(arr_percore)
        return jax.device_put(a.reshape(a.shape[0] * a.shape[1], *a.shape[2:]),
                              self.sharding)

    def __call__(self, named_args):
        args = [named_args[n] for n in self.in_names]
        for z in self.zero_outs:
            zz = np.zeros((self.n_cores * z.shape[0], *z.shape[1:]), z.dtype)
            args.append(jax.device_put(zz, self.sharding))
        outs = self.fn(*args)
        return dict(zip(self.out_names, outs))


# ---------------------------------------------------------------- host side
_CACHE = {}


def make_inputs(x, edge_index, batch, W1, a1s, a1d, b1, W2, a2s, a2d, cfg, ep):
    n, ncores, npc = cfg["n"], cfg["ncores"], cfg["npc"]
    nband, shard, ng = cfg["nband"], cfg["shard"], cfg["ngraph"]
    nt = shard * ncores
    wf, w2c = build_weights(W1, a1s, a1d, W2, a2s, a2d)

    # xts: per-core xT columns in degree-sorted slot order (dummies -> 0)
    xtp = np.zeros((P, n + 1), np.float32)
    xtp[:, :n] = np.asarray(x, np.float32).T
    xtp = xtp.astype(BF16)
    slot_node = ep["slot_node"]                       # [ncores, nslot]
    sidx = np.where(slot_node >= 0, slot_node, n)
    xts = np.stack([xtp[:, sidx[c]] for c in range(ncores)])

    batch = np.asarray(batch, np.int64)
    bl_flat = np.where(slot_node >= 0,
                       batch[np.maximum(slot_node, 0)], 200)
    bl = np.ascontiguousarray(
        bl_flat.reshape(ncores, nband, P).transpose(0, 2, 1)).astype(BF16)

    rep = lambda a: np.broadcast_to(a, (ncores, *a.shape)).copy()
    iog = np.broadcast_to(np.arange(ng, dtype=np.float32).astype(BF16),
                          (P, ng)).copy()
    b1b = np.broadcast_to(np.asarray(b1, np.float32), (P, 64)).copy()
    ident = np.eye(P, dtype=np.float32)

    return {
        "xts": xts,
        "wf": rep(wf.astype(BF16)),
        "w2c": rep(w2c.astype(BF16)),
        "srci": ep["srcidx"],
        "bli": bl,
        "iog": rep(iog),
        "b1i": rep(b1b),
        "idi": rep(ident),
    }


def finish(pool_parts, b2, ng):
    acc = pool_parts.astype(np.float64).sum(axis=0)
    sums = acc[:, :10]
    cnts = np.maximum(acc[:, 10], 1.0)
    pooled = (sums / cnts[:, None] + np.asarray(b2, np.float64)).astype(np.float32)
    m = pooled.max(axis=1, keepdims=True)
    z = pooled - m
    return (z - np.log(np.exp(z).sum(axis=1, keepdims=True))).astype(np.float32)


def kernel(x, edge_index, batch, W1, att_src1, att_dst1, b1,
           W2, att_src2, att_dst2, b2):
    cfg = FULL
    ep = prep_edges(edge_index, cfg)
    key = (tuple(ep["L"]), bool(np.any(np.asarray(b1))))
    if key not in _CACHE:
        nc = build_module(cfg, ep, use_cc=True,
                          has_b1=bool(np.any(np.asarray(b1))))
        _CACHE[key] = Launcher(nc, cfg["ncores"])
    lau = _CACHE[key]

    named = make_inputs(x, edge_index, batch, W1, att_src1, att_dst1, b1,
                        W2, att_src2, att_dst2, cfg, ep)
    named = {k: lau.put(v) for k, v in named.items()}
    outs = lau(named)
    pool = np.asarray(outs["pool"]).reshape(cfg["ncores"], cfg["ngraph"], 12)
    return finish(pool, b2, cfg["ngraph"])
